# revision 7
# baseline (speedup 1.0000x reference)
"""Trainium2 Bass kernel for nn_CustomLoss_7060926235048.

Computes 1 - mean(dice) for the surface-dice loss of reference.py on 8
NeuronCores, depth-sharded, with host-side final reduction.

Math notes:
  - area table == 0.5*min(popcount, 8-popcount), so every target-derived
    quantity is a function of s = 2x2x2 window-sum of targets:
       gt_fg = 1[s==8], gt_bg = 1[s==0], gt_area = 2 - 0.5*|s-4|,
       gt_surf*gt_area == gt_area.
  - fg = exp(window_sum(log_sigmoid(x))); we work with negated fields
       Abar = softplus(-x) = Ln(Exp(-x)+1)   (single ACT table set),
       Bbar = softplus(x) = Abar + x,
    so fg = exp(-SA), bg = exp(-SB) with SA/SB the window sums of Abar/Bbar.
  - masks as ACT exponentials: m8 = exp(1000*s - 8000) in {1, ~0};
    m0 = exp(-1000*s).
  - Sums needed per sample: N1=sum(fg), N2=sum(bg), M8=sum(m8), M0=sum(m0),
    P8=sum(fg*m8), P0=sum(bg*m0), V=sum(|s-4|), FgU=sum(fg*|s-4|),
    BgU=sum(bg*|s-4|).  Then with Nw = number of windows, a = 2-0.5u:
       sum(a)      = 2*Nw - 0.5*V
       sum(fg*a)   = 2*N1 - 0.5*FgU   (same for bg)
       sum(surf*a) = sum(a) - sum(fg*a) - sum(bg*a)
"""

import numpy as np

B = 2
D = 128
H = 256
W = 256
NCORES = 8
NSLAB = 17  # input depth slabs per core
ND = 16  # output depths per core
NQ = 9  # quantities per (b, d')
NCOLS = B * ND * NQ  # collector columns
WP = W - 1  # 255, W-pass output width
EPS = 1e-5

# per-(b,d') pollution constants from the zeroed psum row (partition 127 of
# the h'>=128 block): S=0 there for all three fields -> fg=bg=1, m0=1,
# m8=exp(-8000)=0, u=4, over WP elements.
_Q_NAMES = ["N1", "M8", "P8", "N2", "M0", "P0", "V", "FgU", "BgU"]
_POLLUTION = np.array([WP, 0.0, 0.0, WP, WP, WP, 4.0 * WP, 4.0 * WP, 4.0 * WP])

_CACHE = {}


def _band_matrices():
    """lhsT matrices for the fused H+D pass (out[j] = sum_k lhsT[k,j]*y[k])."""
    bmain = np.zeros((128, 128), np.float32)
    for j in range(128):
        bmain[j, j] = 1.0
        if j + 1 < 128:
            bmain[j + 1, j] = 1.0
    e127 = np.zeros((128, 128), np.float32)
    e127[0, 127] = 1.0  # z[127] += y_blk1[0]
    b1 = bmain.copy()
    b1[:, 127] = 0.0  # blk1 partition 127 stays written-zero
    return np.stack([bmain, e127, b1], axis=1)  # [128, 3, 128]


def _build_program():
    import concourse.bass as bass
    import concourse.mybir as mybir
    import concourse.tile as tile
    from concourse.vector_clock import ScopedClock, VectorClock

    # -- workaround: this walrus build rejects the Tile tail Drain when it
    # carries more than one sem wait; put the waits on sync-engine NOPs.
    def _patched_drain_and_barrier(self, tick_clock, wait_clock):
        gc = tick_clock.global_clock
        for scope, vclock in ScopedClock({None: gc}).items():
            n = len(vclock)
            for p in range(n):
                t = vclock[p]
                if t > 0:
                    vec = [0] * n
                    vec[p] = t
                    nop = self.nc.sync.nop(hint=f"drain_wait_p{p}", nofuse=True)
                    wait_clock.add_sem_waits(
                        nop.ins, ScopedClock({scope: VectorClock(vec)})
                    )
        self.nc.sync.drain()
        self.nc.all_engine_barrier()
        assert self.sems is not None
        popped = self.nc._tile_sem_poison_stack.pop()
        assert popped is self._sem_poison
        self.nc.clear_and_free_semaphores(list(self.sems.allocated().values()))
        self.nc.all_engine_barrier()

    tile.TileContext._drain_and_barrier = _patched_drain_and_barrier

    def _split_multiwait(nc):
        """This walrus build lowers at most ONE sync wait per instruction
        ("Too many sync wait commands").  Move extra waits onto NOPs placed
        immediately before the instruction on the same engine stream."""
        uid = [0]
        for fn in nc.m.functions:
            for bb in fn.blocks:
                insts = bb.instructions
                out = []
                for inst in insts:
                    si = inst.sync_info
                    if si is not None and si.on_wait and len(si.on_wait) > 1:
                        waits = list(si.on_wait)
                        for w in waits[:-1]:
                            nop = mybir.InstNoOp(
                                name=f"mwsplit_{uid[0]}", ins=[], outs=[]
                            )
                            uid[0] += 1
                            nop.engine = inst.engine
                            nop.sync_info = mybir.SyncInfo(
                                on_wait=[w], on_update=[]
                            )
                            out.append(nop)
                        si.on_wait = waits[-1:]
                    out.append(inst)
                insts[:] = out

    f32 = mybir.dt.float32
    AF = mybir.ActivationFunctionType
    OP = mybir.AluOpType

    nc = bass.Bass("TRN2", target_bir_lowering=False, debug=False)
    preds_d = nc.dram_tensor("preds", [B, NSLAB, H, W], f32, kind="ExternalInput")
    targs_d = nc.dram_tensor("targets", [B, NSLAB, H, W], f32, kind="ExternalInput")
    bands_d = nc.dram_tensor("bands", [128, 3, 128], f32, kind="ExternalInput")
    out_d = nc.dram_tensor("partials", [128, NCOLS], f32, kind="ExternalOutput")

    # slab s -> (block j = s//2, sub = s%2); last block holds 1 slab
    nblk = (NSLAB + 1) // 2
    blk_size = [2] * (NSLAB // 2) + ([1] if NSLAB % 2 else [])

    with tile.TileContext(nc) as tc:
        with (
            tc.tile_pool(name="const", bufs=1) as cpool,
            tc.tile_pool(name="inp", bufs=3) as ipool,
            tc.tile_pool(name="work", bufs=3) as wpool,
            tc.tile_pool(name="ypool", bufs=4) as ypool,
            tc.tile_pool(name="fin", bufs=2) as fpool,
            tc.tile_pool(name="psum", bufs=2, space="PSUM") as ppool,
        ):
            bands = cpool.tile([128, 3, 128], f32)
            nc.sync.dma_start(bands[:], bands_d.ap())
            collector = cpool.tile([128, NCOLS], f32)
            bias_c = {}
            for v in (1.0, -4.0, -8000.0, 0.0):
                bias_c[v] = cpool.tile([128, 1], f32, name=f"bias{v}", tag=f"bias{v}")
                nc.vector.memset(bias_c[v][:], v)

            for b in range(B):
                ytiles = {}  # j -> (YA, YB, YT)

                def make_block(j, b=b):
                    ns = blk_size[j]
                    x = ipool.tile([128, ns, 2, W], f32, tag="x")
                    t = ipool.tile([128, ns, 2, W], f32, tag="t")
                    src = preds_d.ap()[b, 2 * j : 2 * j + ns].rearrange(
                        "s (hb p) w -> p s hb w", p=128
                    )
                    nc.sync.dma_start(x[:], src)
                    tsrc = targs_d.ap()[b, 2 * j : 2 * j + ns].rearrange(
                        "s (hb p) w -> p s hb w", p=128
                    )
                    nc.sync.dma_start(t[:], tsrc)
                    e = wpool.tile([128, ns, 2, W], f32, tag="e")
                    ab = wpool.tile([128, ns, 2, W], f32, tag="ab")
                    bb = wpool.tile([128, ns, 2, W], f32, tag="bb")
                    nc.scalar.activation(e[:], x[:], AF.Exp, bias=bias_c[0.0][:], scale=-1.0)
                    nc.scalar.activation(ab[:], e[:], AF.Ln, bias=bias_c[1.0][:])
                    nc.vector.tensor_add(bb[:], ab[:], x[:])
                    ya = ypool.tile([128, ns, 2, WP], f32, tag="ya")
                    yb = ypool.tile([128, ns, 2, WP], f32, tag="yb")
                    yt = ypool.tile([128, ns, 2, WP], f32, tag="yt")
                    for src_t, dst_t in ((ab, ya), (bb, yb), (t, yt)):
                        nc.vector.tensor_add(
                            dst_t[:],
                            src_t[:, :, :, 0:WP],
                            src_t[:, :, :, 1:W],
                        )
                    return (ya, yb, yt)

                for dp in range(ND):
                    for j in {dp // 2, (dp + 1) // 2}:
                        if j not in ytiles:
                            ytiles[j] = make_block(j)

                    # rhs slices for slabs dp and dp+1, per field
                    slabs = []
                    for s in (dp, dp + 1):
                        yy = ytiles[s // 2]
                        slabs.append(tuple(yy[f][:, s % 2] for f in range(3)))

                    sa = ppool.tile([128, 2, WP], f32, tag="sa")
                    sb = ppool.tile([128, 2, WP], f32, tag="sb")
                    st = ppool.tile([128, 2, WP], f32, tag="st")
                    S = (sa, sb, st)
                    # weight-major order: 3 LDWEIGHTS per d'
                    # blk0 (h' 0..127): Bmain on yb0 then E127 on yb1
                    for f in range(3):
                        for si in range(2):
                            nc.tensor.matmul(
                                S[f][:, 0, :],
                                bands[:, 0, :],
                                slabs[si][f][:, 0, :],
                                start=(si == 0),
                                stop=False,
                            )
                    for f in range(3):
                        for si in range(2):
                            nc.tensor.matmul(
                                S[f][:, 0, :],
                                bands[:, 1, :],
                                slabs[si][f][:, 1, :],
                                start=False,
                                stop=(si == 1),
                            )
                    # blk1 (h' 128..254 + zero row): B1 on yb1
                    for f in range(3):
                        for si in range(2):
                            nc.tensor.matmul(
                                S[f][:, 1, :],
                                bands[:, 2, :],
                                slabs[si][f][:, 1, :],
                                start=(si == 0),
                                stop=(si == 1),
                            )

                    def col(q, dp=dp, b=b):
                        c = b * ND * NQ + dp * NQ + q
                        return collector[:, c : c + 1]

                    fg = fpool.tile([128, 2, WP], f32, tag="fg")
                    bg = fpool.tile([128, 2, WP], f32, tag="bg")
                    m8 = fpool.tile([128, 2, WP], f32, tag="m8")
                    m0 = fpool.tile([128, 2, WP], f32, tag="m0")
                    u = fpool.tile([128, 2, WP], f32, tag="u")
                    nc.scalar.activation(
                        fg[:], sa[:], AF.Exp, bias=bias_c[0.0][:], scale=-1.0,
                        accum_out=col(0),
                    )
                    nc.scalar.activation(
                        m8[:], st[:], AF.Exp, scale=1000.0,
                        bias=bias_c[-8000.0][:], accum_out=col(1),
                    )
                    nc.scalar.activation(
                        bg[:], sb[:], AF.Exp, bias=bias_c[0.0][:], scale=-1.0,
                        accum_out=col(3),
                    )
                    nc.scalar.activation(
                        m0[:], st[:], AF.Exp, bias=bias_c[0.0][:],
                        scale=-1000.0, accum_out=col(4),
                    )
                    nc.scalar.activation(
                        u[:], st[:], AF.Abs, bias=bias_c[-4.0][:],
                        accum_out=col(6),
                    )
                    for q, (t0, t1, tag) in (
                        (2, (fg, m8, "p8o")),
                        (5, (bg, m0, "p0o")),
                        (7, (fg, u, "fuo")),
                        (8, (bg, u, "buo")),
                    ):
                        scr = fpool.tile([128, 2, WP], f32, tag=tag)
                        nc.vector.scalar_tensor_tensor(
                            scr[:], t0[:], 0.0, t1[:], OP.bypass, OP.mult,
                            accum_out=col(q),
                        )

                    # free blocks no longer needed
                    done = dp // 2
                    if dp + 1 < ND and (dp + 1) // 2 > done and done in ytiles:
                        del ytiles[done]

            nc.sync.dma_start(out_d.ap(), collector[:])

    _split_multiwait(nc)
    return nc


def _host_reduce(partials_list):
    """partials_list: per-core [128, NCOLS] arrays -> final loss (np.float32)."""
    acc = np.zeros((B, NQ), np.float64)
    n_retained = 0
    for k, p in enumerate(partials_list):
        q = p.astype(np.float64).sum(axis=0).reshape(B, ND, NQ)
        keep = slice(1, None) if k == NCORES - 1 else slice(None)
        q = q[:, keep, :]
        acc += q.sum(axis=1)
        n_retained += q.shape[1]
    assert n_retained == D - 1
    acc -= (D - 1) * _POLLUTION[None, :]

    N1, M8, P8, N2, M0, P0, V, FgU, BgU = [acc[:, i] for i in range(NQ)]
    nw = float((D - 1) * (H - 1) * (W - 1))
    fg_dice = (2.0 * P8 + EPS) / (N1 + M8 + EPS)
    bg_dice = (2.0 * P0 + EPS) / (N2 + M0 + EPS)
    sum_a = 2.0 * nw - 0.5 * V
    surf_a = sum_a - (2.0 * N1 - 0.5 * FgU) - (2.0 * N2 - 0.5 * BgU)
    surf_dice = (2.0 * surf_a + EPS) / (surf_a + sum_a + EPS)
    dice = (fg_dice + bg_dice + surf_dice) / 3.0
    return np.float32(1.0 - dice.mean())


def _shard_inputs(preds, targets):
    """Full [B,1,D,H,W] -> per-core in_maps."""
    p = np.ascontiguousarray(preds.reshape(B, D, H, W), dtype=np.float32)
    t = np.ascontiguousarray(targets.reshape(B, D, H, W), dtype=np.float32)
    bands = np.ascontiguousarray(_band_matrices(), dtype=np.float32)
    in_maps = []
    for k in range(NCORES):
        d0 = 16 * k if k < NCORES - 1 else D - NSLAB
        in_maps.append(
            {
                "preds": np.ascontiguousarray(p[:, d0 : d0 + NSLAB]),
                "targets": np.ascontiguousarray(t[:, d0 : d0 + NSLAB]),
                "bands": bands,
            }
        )
    return in_maps


def kernel(preds, targets, power, kernel, area):
    from concourse.bass_utils import run_bass_kernel_spmd

    preds = np.asarray(preds)
    targets = np.asarray(targets)
    in_maps = _shard_inputs(preds, targets)
    if "nc" not in _CACHE:
        _CACHE["nc"] = _build_program()
    res = run_bass_kernel_spmd(_CACHE["nc"], in_maps, core_ids=list(range(NCORES)))
    return _host_reduce([r["partials"] for r in res.results])


# revision 15
# speedup vs baseline: 3.1499x; 3.1499x over previous
"""Trainium2 Bass kernel for nn_CustomLoss_7060926235048.

Computes 1 - mean(dice) for the surface-dice loss of reference.py on 8
NeuronCores, depth-sharded, with host-side final reduction.

Math notes:
  - area table == 0.5*min(popcount, 8-popcount), so every target-derived
    quantity is a function of s = 2x2x2 window-sum of targets:
       gt_fg = 1[s==8], gt_bg = 1[s==0], gt_area = 2 - 0.5*|s-4|,
       gt_surf*gt_area == gt_area.
  - fg = exp(window_sum(log_sigmoid(x))); we work with the negated field
       Abar = softplus(-x) = Ln(Exp(-x)+1)   (single ACT table set),
    so fg = exp(-SA).  Bbar = softplus(x) = Abar + x, and by linearity
    SB = window_sum(Bbar) = SA + window_sum(x): the SB PSUM accumulates the
    banded matmuls of BOTH the Abar field and the raw-preds field.
  - The 2x2x2 window sum is separable: W-pair-sums on DVE (fp16 out), then
    the H-pair-sum and the D-pair-sum are fused into banded-matrix matmuls
    accumulating in PSUM (fp32).
  - Sums per (b, d'): N1=sum(fg), N2=sum(bg), M8=sum(1[s=8]),
    UM=sum(1[|s-4|=4])=M8+M0, P8=sum(fg*1[s=8]), P0=sum(bg*1[s=0]),
    V=sum(|s-4|), FgU=sum(fg*|s-4|), BgU=sum(bg*|s-4|).
    With Nw windows and a = 2-0.5u:
       sum(a)      = 2*Nw - 0.5*V
       sum(fg*a)   = 2*N1 - 0.5*FgU   (same for bg)
       sum(surf*a) = sum(a) - sum(fg*a) - sum(bg*a)
"""

import numpy as np

B = 2
D = 128
H = 256
W = 256
NCORES = 8
NSLAB = 17  # input depth slabs per core
ND = 16  # output depths per core
NQ = 9  # quantities per (b, d')
NCOLS = B * ND * NQ  # collector columns
WP = W - 1  # 255, W-pass output width
EPS = 1e-5

# Collector column semantics (per (b, d')):
#   0:N1  1:M8  2:P8  3:N2  4:UM(=M8+M0)  5:P0  6:V  7:FgU  8:BgU
# Pollution constants from the zeroed psum row (partition 127 of the
# h'>=128 block): S=0 there for all fields -> fg=bg=1, u=4, 1[s=0]=1,
# 1[s=8]=0, 1[u=4]=1, over WP elements.
_Q_NAMES = ["N1", "M8", "P8", "N2", "UM", "P0", "V", "FgU", "BgU"]
_POLLUTION = np.array([WP, 0.0, 0.0, WP, WP, WP, 4.0 * WP, 4.0 * WP, 4.0 * WP])

_CACHE = {}
REPEAT = 1  # timing experiments only


def _band_matrices():
    """lhsT matrices for the fused H+D pass (out[j] = sum_k lhsT[k,j]*y[k])."""
    bmain = np.zeros((128, 128), np.float32)
    for j in range(128):
        bmain[j, j] = 1.0
        if j + 1 < 128:
            bmain[j + 1, j] = 1.0
    e127 = np.zeros((128, 128), np.float32)
    e127[0, 127] = 1.0  # z[127] += y_blk1[0]
    b1 = bmain.copy()
    b1[:, 127] = 0.0  # blk1 partition 127 stays written-zero
    return np.stack([bmain, e127, b1], axis=1)  # [128, 3, 128]


def _build_program(split_multiwait=True):
    import concourse.bass as bass
    import concourse.mybir as mybir
    import concourse.tile as tile
    from concourse.vector_clock import ScopedClock, VectorClock

    # -- workaround: this walrus build rejects instructions carrying more
    # than one sem wait ("Too many sync wait commands").
    def _patched_drain_and_barrier(self, tick_clock, wait_clock):
        gc = tick_clock.global_clock
        for scope, vclock in ScopedClock({None: gc}).items():
            n = len(vclock)
            for p in range(n):
                t = vclock[p]
                if t > 0:
                    vec = [0] * n
                    vec[p] = t
                    nop = self.nc.sync.nop(hint=f"drain_wait_p{p}", nofuse=True)
                    wait_clock.add_sem_waits(
                        nop.ins, ScopedClock({scope: VectorClock(vec)})
                    )
        self.nc.sync.drain()
        self.nc.all_engine_barrier()
        assert self.sems is not None
        popped = self.nc._tile_sem_poison_stack.pop()
        assert popped is self._sem_poison
        self.nc.clear_and_free_semaphores(list(self.sems.allocated().values()))
        self.nc.all_engine_barrier()

    tile.TileContext._drain_and_barrier = _patched_drain_and_barrier

    def _split_multiwait(nc):
        """Move extra waits onto NOPs placed immediately before the
        instruction on the same engine stream (walrus allows one wait)."""
        uid = [0]
        for fn in nc.m.functions:
            for bb in fn.blocks:
                insts = bb.instructions
                out = []
                for inst in insts:
                    si = inst.sync_info
                    if si is not None and si.on_wait and len(si.on_wait) > 1:
                        waits = list(si.on_wait)
                        for w in waits[:-1]:
                            nop = mybir.InstNoOp(
                                name=f"mwsplit_{uid[0]}", ins=[], outs=[]
                            )
                            uid[0] += 1
                            nop.engine = inst.engine
                            nop.sync_info = mybir.SyncInfo(
                                on_wait=[w], on_update=[]
                            )
                            out.append(nop)
                        si.on_wait = waits[-1:]
                    out.append(inst)
                insts[:] = out

    f32 = mybir.dt.float32
    f16 = mybir.dt.float16
    AF = mybir.ActivationFunctionType
    OP = mybir.AluOpType

    nc = bass.Bass("TRN2", target_bir_lowering=False, debug=False)
    preds_d = nc.dram_tensor("preds", [B, NSLAB, H, W], f32, kind="ExternalInput")
    targs_d = nc.dram_tensor("targets", [B, NSLAB, H, W], f32, kind="ExternalInput")
    bands_d = nc.dram_tensor("bands", [128, 3, 128], f16, kind="ExternalInput")
    out_d = nc.dram_tensor("partials", [128, NCOLS], f32, kind="ExternalOutput")

    # slab s -> (block j = s//2, sub = s%2); last block holds 1 slab
    blk_size = [2] * (NSLAB // 2) + ([1] if NSLAB % 2 else [])

    with tile.TileContext(nc) as tc:
        with (
            tc.tile_pool(name="const", bufs=1) as cpool,
            tc.tile_pool(name="inp", bufs=3) as ipool,
            tc.tile_pool(name="work", bufs=3) as wpool,
            tc.tile_pool(name="ypool", bufs=4) as ypool,
            tc.tile_pool(name="fin", bufs=2) as fpool,
            tc.tile_pool(name="psum", bufs=1, space="PSUM") as ppool,
        ):
            bands = cpool.tile([128, 3, 128], f16)
            nc.sync.dma_start(bands[:], bands_d.ap())
            collector = cpool.tile([128, NCOLS], f32)
            bias_c = {}
            for v in (1.0, -4.0, 0.0):
                bias_c[v] = cpool.tile([128, 1], f32, name=f"bias{v}", tag=f"bias{v}")
                nc.vector.memset(bias_c[v][:], v)

            for _rep in range(REPEAT):
              for b in range(B):
                ytiles = {}  # j -> (YA, YX, YT)

                def make_block(j, b=b):
                    ns = blk_size[j]
                    x = ipool.tile([128, ns, 2, W], f32, tag="x")
                    t = ipool.tile([128, ns, 2, W], f32, tag="t")
                    src = preds_d.ap()[b, 2 * j : 2 * j + ns].rearrange(
                        "s (hb p) w -> p s hb w", p=128
                    )
                    nc.sync.dma_start(x[:], src)
                    tsrc = targs_d.ap()[b, 2 * j : 2 * j + ns].rearrange(
                        "s (hb p) w -> p s hb w", p=128
                    )
                    nc.sync.dma_start(t[:], tsrc)
                    e = wpool.tile([128, ns, 2, W], f32, tag="e")
                    ab = wpool.tile([128, ns, 2, W], f32, tag="ab")
                    nc.scalar.activation(
                        e[:], x[:], AF.Exp, bias=bias_c[0.0][:], scale=-1.0
                    )
                    nc.scalar.activation(ab[:], e[:], AF.Ln, bias=bias_c[1.0][:])
                    ya = ypool.tile([128, ns, 2, WP], f16, tag="ya")
                    yx = ypool.tile([128, ns, 2, WP], f16, tag="yx")
                    yt = ypool.tile([128, ns, 2, WP], f16, tag="yt")
                    for src_t, dst_t, eng in (
                        (ab, ya, nc.vector),
                        (x, yx, nc.gpsimd),
                        (t, yt, nc.gpsimd),
                    ):
                        eng.tensor_add(
                            dst_t[:],
                            src_t[:, :, :, 0:WP],
                            src_t[:, :, :, 1:W],
                        )
                    return (ya, yx, yt)

                for pp in range(ND // 2):
                    dp0 = 2 * pp
                    need = {dp0 // 2, (dp0 + 1) // 2, (dp0 + 2) // 2}
                    for j in sorted(need):
                        if j not in ytiles:
                            ytiles[j] = make_block(j)

                    def yslab(s, f):
                        return ytiles[s // 2][f][:, s % 2]

                    # psum: [128, pair-quarter, hb, WP] = 2 banks per tag
                    # quarters padded to 256 floats so each (q, hb) region
                    # stays inside one psum bank
                    sa = ppool.tile([128, 2, 2, W], f32, tag="sa")
                    sb = ppool.tile([128, 2, 2, W], f32, tag="sb")
                    st = ppool.tile([128, 2, 2, W], f32, tag="st")

                    # (weight_idx, region(out AP), rhs) specs; SB gets both
                    # the Abar field and the raw-preds field (SB = SA + SX).
                    mms = []
                    for q in range(2):
                        dp = dp0 + q
                        for s in (dp, dp + 1):
                            # blk0: Bmain on b0 rows, E127 cross on b1 rows
                            mms.append((0, ("sa", q, 0), yslab(s, 0)[:, 0]))
                            mms.append((1, ("sa", q, 0), yslab(s, 0)[:, 1]))
                            mms.append((2, ("sa", q, 1), yslab(s, 0)[:, 1]))
                            for f in (0, 1):
                                mms.append((0, ("sb", q, 0), yslab(s, f)[:, 0]))
                                mms.append((1, ("sb", q, 0), yslab(s, f)[:, 1]))
                                mms.append((2, ("sb", q, 1), yslab(s, f)[:, 1]))
                            mms.append((0, ("st", q, 0), yslab(s, 2)[:, 0]))
                            mms.append((1, ("st", q, 0), yslab(s, 2)[:, 1]))
                            mms.append((2, ("st", q, 1), yslab(s, 2)[:, 1]))
                    mms.sort(key=lambda m: m[0])  # weight-major, stable
                    S = {"sa": sa, "sb": sb, "st": st}
                    nreg = {}
                    for widx, reg, rhs in mms:
                        nreg[reg] = nreg.get(reg, 0) + 1
                    seen = {}
                    for widx, reg, rhs in mms:
                        seen[reg] = seen.get(reg, 0) + 1
                        tag, q, hb = reg
                        nc.tensor.matmul(
                            S[tag][:, q, hb, 0:WP],
                            bands[:, widx, :],
                            rhs,
                            start=(seen[reg] == 1),
                            stop=(seen[reg] == nreg[reg]),
                        )

                    def col(q_, dp_, b=b):
                        c = b * ND * NQ + dp_ * NQ + q_
                        return collector[:, c : c + 1]

                    def cols2(q_):
                        # accum for the two d' of the pair is separate ops,
                        # so return per-quarter column APs
                        return (col(q_, dp0), col(q_, dp0 + 1))

                    fg = fpool.tile([128, 2, 2, WP], f16, tag="fg")
                    bg = fpool.tile([128, 2, 2, WP], f16, tag="bg")
                    u = fpool.tile([128, 2, 2, WP], f16, tag="u")
                    # per-quarter ACT ops (accum_out is per (b,d') column)
                    for q in range(2):
                        dp = dp0 + q
                        nc.scalar.activation(
                            fg[:, q], sa[:, q, :, 0:WP], AF.Exp, bias=bias_c[0.0][:],
                            scale=-1.0, accum_out=col(0, dp),
                        )
                        nc.scalar.activation(
                            bg[:, q], sb[:, q, :, 0:WP], AF.Exp, bias=bias_c[0.0][:],
                            scale=-1.0, accum_out=col(3, dp),
                        )
                        nc.scalar.activation(
                            u[:, q], st[:, q, :, 0:WP], AF.Abs, bias=bias_c[-4.0][:],
                            accum_out=col(6, dp),
                        )
                        # P8 = sum (s==8)*fg ; P0 = sum (s==0)*bg ; M8
                        p8o = fpool.tile([128, 2, WP], f16, tag="p8o")
                        p0o = fpool.tile([128, 2, WP], f16, tag="p0o")
                        m8o = fpool.tile([128, 2, WP], f16, tag="m8o")
                        nc.vector.scalar_tensor_tensor(
                            p8o[:], st[:, q, :, 0:WP], 8.0, fg[:, q],
                            OP.is_equal, OP.mult, accum_out=col(2, dp),
                        )
                        nc.vector.scalar_tensor_tensor(
                            p0o[:], st[:, q, :, 0:WP], 0.0, bg[:, q],
                            OP.is_equal, OP.mult, accum_out=col(5, dp),
                        )
                        nc.vector.tensor_scalar(
                            m8o[:], st[:, q, :, 0:WP], 8.0, None,
                            OP.is_equal, OP.add, accum_out=col(1, dp),
                        )
                        # POOL: UM = sum 1[u==4]; FgU; BgU  (SBUF fp16)
                        umo = fpool.tile([128, 2, WP], f16, tag="umo")
                        fuo = fpool.tile([128, 2, WP], f16, tag="fuo")
                        buo = fpool.tile([128, 2, WP], f16, tag="buo")
                        nc.vector.tensor_scalar(
                            umo[:], u[:, q], 4.0, None,
                            OP.is_equal, OP.add, accum_out=col(4, dp),
                        )
                        nc.vector.scalar_tensor_tensor(
                            fuo[:], fg[:, q], 0.0, u[:, q],
                            OP.bypass, OP.mult, accum_out=col(7, dp),
                        )
                        nc.vector.scalar_tensor_tensor(
                            buo[:], bg[:, q], 0.0, u[:, q],
                            OP.bypass, OP.mult, accum_out=col(8, dp),
                        )

                    # free blocks no longer needed
                    for jdone in list(ytiles):
                        if jdone < (dp0 + 2) // 2 and jdone not in need:
                            del ytiles[jdone]

            nc.sync.dma_start(out_d.ap(), collector[:])

    if split_multiwait:
        _split_multiwait(nc)
    return nc


def _host_reduce(partials_list):
    """partials_list: per-core [128, NCOLS] arrays -> final loss (np.float32)."""
    acc = np.zeros((B, NQ), np.float64)
    n_retained = 0
    for k, p in enumerate(partials_list):
        q = p.astype(np.float64).sum(axis=0).reshape(B, ND, NQ)
        keep = slice(1, None) if k == NCORES - 1 else slice(None)
        q = q[:, keep, :]
        acc += q.sum(axis=1)
        n_retained += q.shape[1]
    assert n_retained == D - 1
    acc -= (D - 1) * _POLLUTION[None, :]

    N1, M8, P8, N2, UM, P0, V, FgU, BgU = [acc[:, i] for i in range(NQ)]
    M0 = UM - M8
    nw = float((D - 1) * (H - 1) * (W - 1))
    fg_dice = (2.0 * P8 + EPS) / (N1 + M8 + EPS)
    bg_dice = (2.0 * P0 + EPS) / (N2 + M0 + EPS)
    sum_a = 2.0 * nw - 0.5 * V
    surf_a = sum_a - (2.0 * N1 - 0.5 * FgU) - (2.0 * N2 - 0.5 * BgU)
    surf_dice = (2.0 * surf_a + EPS) / (surf_a + sum_a + EPS)
    dice = (fg_dice + bg_dice + surf_dice) / 3.0
    return np.float32(1.0 - dice.mean())


def _shard_inputs(preds, targets):
    """Full [B,1,D,H,W] -> per-core in_maps."""
    p = np.ascontiguousarray(preds.reshape(B, D, H, W), dtype=np.float32)
    t = np.ascontiguousarray(targets.reshape(B, D, H, W), dtype=np.float32)
    bands = np.ascontiguousarray(_band_matrices(), dtype=np.float16)
    in_maps = []
    for k in range(NCORES):
        d0 = 16 * k if k < NCORES - 1 else D - NSLAB
        in_maps.append(
            {
                "preds": np.ascontiguousarray(p[:, d0 : d0 + NSLAB]),
                "targets": np.ascontiguousarray(t[:, d0 : d0 + NSLAB]),
                "bands": bands,
            }
        )
    return in_maps


def kernel(preds, targets, power, kernel, area):
    from concourse.bass_utils import run_bass_kernel_spmd

    preds = np.asarray(preds)
    targets = np.asarray(targets)
    in_maps = _shard_inputs(preds, targets)
    if "nc" not in _CACHE:
        _CACHE["nc"] = _build_program()
    res = run_bass_kernel_spmd(_CACHE["nc"], in_maps, core_ids=list(range(NCORES)))
    return _host_reduce([r["partials"] for r in res.results])
